# revision 1
# baseline (speedup 1.0000x reference)
"""KGCN (2-hop, 16-neighbor, relation-attention GNN) forward on 8 Trainium2 NeuronCores.

Strategy (per sharding hint): data-parallel over the batch dim. Each of the 8
cores gets 512 of the 4096 batch rows; the entity/relation embedding tables,
adjacency tables and aggregator weights are replicated to every core. All
gathers (adjacency expansion + embedding lookups) run on-device via SWDGE
indirect DMA — one index per partition per instruction (the HW contract:
each partition's descriptor reads a contiguous run starting at its index).
Relation-attention softmax, neighbor aggregation, the 64x64 linear +
activation and the final user.item scores run on DVE/ACT/PE.

Layouts:
  - batch-on-partition for gathers + neighbor aggregation ([128, ...] tiles,
    4 b-tiles per core)
  - feature-on-partition ([64, tokens]) for the W matmul, entered/exited with
    PE transposes
  - relation attention: exp(<user, rel_r>) for all 32 relations is computed
    once per batch row as a [32, 512] matmul + Exp, transposed to [128, 32]
    per b-tile, and per-(b,m,n) scores are selected on DVE with a 32-step
    one-hot accumulate over the relation ids.
"""

import sys

sys.path.insert(0, "/opt/trn_rl_repo")

from contextlib import ExitStack

import numpy as np

import concourse.bass as bass
import concourse.mybir as mybir
import concourse.tile as tile
from concourse import bacc
from concourse.bass_utils import run_bass_kernel_spmd
from concourse.masks import make_identity

F32 = mybir.dt.float32
I32 = mybir.dt.int32
AF = mybir.ActivationFunctionType
ALU = mybir.AluOpType

N_CORES = 8
BATCH = 4096
BL = BATCH // N_CORES  # 512 batch rows per core
P = 128  # partitions
NT = BL // P  # 4 b-tiles per core
K = 16  # neighbors per node
D = 64  # embedding dim
R = 32  # num relations
TOTAL = 110000  # entity table rows (users + entities)


def build_program(total=TOTAL, bl=BL):
    nt = bl // P
    nc = bacc.Bacc(None, target_bir_lowering=False)

    u_d = nc.dram_tensor("u32", [bl], I32, kind="ExternalInput")
    v_d = nc.dram_tensor("v32", [bl], I32, kind="ExternalInput")
    ae_d = nc.dram_tensor("adj_ent32", [total, K], I32, kind="ExternalInput")
    ar_d = nc.dram_tensor("adj_rel32", [total, K], I32, kind="ExternalInput")
    ent_d = nc.dram_tensor("ent", [total, D], F32, kind="ExternalInput")
    relT_d = nc.dram_tensor("relT", [D, R], F32, kind="ExternalInput")
    wt_d = nc.dram_tensor("Wt", [D, D], F32, kind="ExternalInput")
    bias_d = nc.dram_tensor("bias", [D], F32, kind="ExternalInput")
    out_d = nc.dram_tensor("out", [bl], F32, kind="ExternalOutput")

    def gather(out_ap, table_ap, idx_ap):
        # idx_ap must be [P, 1]: one descriptor per partition, reading
        # out_ap's per-partition byte count contiguously from row idx[p].
        nc.gpsimd.indirect_dma_start(
            out=out_ap,
            out_offset=None,
            in_=table_ap,
            in_offset=bass.IndirectOffsetOnAxis(ap=idx_ap, axis=0),
        )

    with ExitStack() as ctx:
        tc = ctx.enter_context(tile.TileContext(nc))
        const = ctx.enter_context(tc.tile_pool(name="const", bufs=1))
        persist = ctx.enter_context(tc.tile_pool(name="persist", bufs=1))
        idxp = ctx.enter_context(tc.tile_pool(name="idxp", bufs=2))
        gat = ctx.enter_context(tc.tile_pool(name="gat", bufs=8))
        work = ctx.enter_context(tc.tile_pool(name="work", bufs=3))
        big = ctx.enter_context(tc.tile_pool(name="big", bufs=2))
        psT = ctx.enter_context(tc.tile_pool(name="psT", bufs=2, space="PSUM"))
        psM = ctx.enter_context(tc.tile_pool(name="psM", bufs=2, space="PSUM"))
        psB = ctx.enter_context(tc.tile_pool(name="psB", bufs=2, space="PSUM"))

        # ---- constants ----
        ident = const.tile([P, P], F32)
        make_identity(nc, ident[:])
        ones64 = const.tile([D, 1], F32)
        nc.vector.memset(ones64[:], 1.0)
        wt_sb = const.tile([D, D], F32)
        nc.sync.dma_start(out=wt_sb[:], in_=wt_d[:])
        relT_sb = const.tile([D, R], F32)
        nc.sync.dma_start(out=relT_sb[:], in_=relT_d[:])
        bias_sb = const.tile([D, 1], F32)
        nc.sync.dma_start(out=bias_sb[:], in_=bias_d.rearrange("(d one) -> d one", one=1))

        # ---- persistent per-b-tile buffers ----
        ev0 = [persist.tile([P, D], F32, name=f"ev0_{i}") for i in range(nt)]
        ev1 = [persist.tile([P, K * D], F32, name=f"ev1_{i}") for i in range(nt)]
        h0 = [persist.tile([P, D], F32, name=f"h0_{i}") for i in range(nt)]
        h1 = [persist.tile([P, K * D], F32, name=f"h1_{i}") for i in range(nt)]
        esc0 = [persist.tile([P, K], F32, name=f"esc0_{i}") for i in range(nt)]
        esc1 = [persist.tile([P, K * K], F32, name=f"esc1_{i}") for i in range(nt)]
        rec0 = [persist.tile([P, 1], F32, name=f"rec0_{i}") for i in range(nt)]
        rec1 = [persist.tile([P, K], F32, name=f"rec1_{i}") for i in range(nt)]
        e2t = [persist.tile([P, K * K], I32, name=f"e2_{i}") for i in range(nt)]
        r0f = [persist.tile([P, K], F32, name=f"r0f_{i}") for i in range(nt)]
        r1f = [persist.tile([P, K * K], F32, name=f"r1f_{i}") for i in range(nt)]
        escb = [persist.tile([P, R], F32, name=f"escb_{i}") for i in range(nt)]
        userT = persist.tile([D, bl], F32, tag="userT")
        x0T = persist.tile([D, bl], F32, tag="x0T")
        xfT = persist.tile([D, bl], F32, tag="xfT")

        # ================= phase 1: indices + embedding gathers =================
        for i in range(nt):
            uidx = idxp.tile([P, 1], I32, tag="uidx")
            nc.sync.dma_start(
                out=uidx[:], in_=u_d[i * P : (i + 1) * P].rearrange("(p one) -> p one", one=1)
            )
            user_g = gat.tile([P, D], F32, tag="user_g")
            gather(user_g[:], ent_d[:], uidx[:, 0:1])
            pst = psT.tile([D, P], F32, tag="pst")
            nc.tensor.transpose(pst[:], user_g[:], ident[:])
            nc.vector.tensor_copy(userT[:, i * P : (i + 1) * P], pst[:])

            vidx = idxp.tile([P, 1], I32, tag="vidx")
            nc.sync.dma_start(
                out=vidx[:], in_=v_d[i * P : (i + 1) * P].rearrange("(p one) -> p one", one=1)
            )
            gather(ev0[i][:], ent_d[:], vidx[:, 0:1])

            e1 = idxp.tile([P, K], I32, tag="e1")
            gather(e1[:], ae_d[:], vidx[:, 0:1])
            r0 = idxp.tile([P, K], I32, tag="r0")
            gather(r0[:], ar_d[:], vidx[:, 0:1])
            nc.vector.tensor_copy(r0f[i][:], r0[:])
            r1 = idxp.tile([P, K * K], I32, tag="r1")
            for n in range(K):
                gather(ev1[i][:, n * D : (n + 1) * D], ent_d[:], e1[:, n : n + 1])
                gather(e2t[i][:, n * K : (n + 1) * K], ae_d[:], e1[:, n : n + 1])
                gather(r1[:, n * K : (n + 1) * K], ar_d[:], e1[:, n : n + 1])
            nc.vector.tensor_copy(r1f[i][:], r1[:])

        # ================= phase 2: relation scores =================
        ps = psM.tile([R, bl], F32, tag="mm")
        nc.tensor.matmul(ps[:], lhsT=relT_sb[:], rhs=userT[:], start=True, stop=True)
        esc_sb = work.tile([R, bl], F32, tag="esc_sb")
        nc.scalar.activation(esc_sb[:], ps[:], AF.Exp)
        for i in range(nt):
            pe = psB.tile([P, R], F32, tag="pe")
            nc.tensor.transpose(pe[:], esc_sb[:, i * P : (i + 1) * P], ident[:R, :R])
            nc.vector.tensor_copy(escb[i][:], pe[:])

        # ======== phase 3: select exp-scores by relation id, denominators ========
        for i in range(nt):
            nc.vector.memset(esc0[i][:], 0.0)
            nc.vector.memset(esc1[i][:], 0.0)
            for r in range(R):
                m0 = work.tile([P, K], F32, tag="m0")
                nc.vector.tensor_scalar(
                    out=m0[:], in0=r0f[i][:], scalar1=float(r), scalar2=None,
                    op0=ALU.is_equal,
                )
                nc.vector.scalar_tensor_tensor(
                    out=esc0[i][:], in0=m0[:], scalar=escb[i][:, r : r + 1],
                    in1=esc0[i][:], op0=ALU.mult, op1=ALU.add,
                )
                m1 = work.tile([P, K * K], F32, tag="m1")
                nc.vector.tensor_scalar(
                    out=m1[:], in0=r1f[i][:], scalar1=float(r), scalar2=None,
                    op0=ALU.is_equal,
                )
                nc.vector.scalar_tensor_tensor(
                    out=esc1[i][:], in0=m1[:], scalar=escb[i][:, r : r + 1],
                    in1=esc1[i][:], op0=ALU.mult, op1=ALU.add,
                )
            den0 = work.tile([P, 1], F32, tag="den0")
            nc.vector.tensor_reduce(
                out=den0[:], in_=esc0[i][:], axis=mybir.AxisListType.X, op=ALU.add
            )
            nc.vector.reciprocal(rec0[i][:], den0[:])
            den1 = work.tile([P, K], F32, tag="den1")
            nc.vector.tensor_reduce(
                out=den1[:],
                in_=esc1[i][:].rearrange("p (m n) -> p m n", n=K),
                axis=mybir.AxisListType.X,
                op=ALU.add,
            )
            nc.vector.reciprocal(rec1[i][:], den1[:])

        # ================= phase 5 (early): iter-0 hop-0 =================
        # x0 = ev0 + softmax(score) . ev1 ; h0 = sigmoid(x0 @ W.T + b)
        for i in range(nt):
            wev = work.tile([P, K, D], F32, tag="wev0")
            nc.vector.tensor_tensor(
                out=wev[:],
                in0=ev1[i][:].rearrange("p (n d) -> p n d", n=K),
                in1=esc0[i][:].broadcast_to([P, K, D]),
                op=ALU.mult,
            )
            agg = work.tile([P, D], F32, tag="agg0")
            nc.vector.tensor_reduce(
                out=agg[:],
                in_=wev[:].rearrange("p n d -> p d n"),
                axis=mybir.AxisListType.X,
                op=ALU.add,
            )
            x0 = work.tile([P, D], F32, tag="x0")
            nc.vector.scalar_tensor_tensor(
                out=x0[:], in0=agg[:], scalar=rec0[i][:, 0:1], in1=ev0[i][:],
                op0=ALU.mult, op1=ALU.add,
            )
            pst = psT.tile([D, P], F32, tag="pst")
            nc.tensor.transpose(pst[:], x0[:], ident[:])
            nc.vector.tensor_copy(x0T[:, i * P : (i + 1) * P], pst[:])

        pm0 = psM.tile([D, bl], F32, tag="mm")
        nc.tensor.matmul(pm0[:], lhsT=wt_sb[:], rhs=x0T[:], start=True, stop=True)
        h0T = work.tile([D, bl], F32, tag="h0T")
        nc.scalar.activation(h0T[:], pm0[:], AF.Sigmoid, bias=bias_sb[:, 0:1])
        for i in range(nt):
            pbt = psB.tile([P, D], F32, tag="pbt")
            nc.tensor.transpose(pbt[:], h0T[:, i * P : (i + 1) * P], ident[:D, :D])
            nc.vector.tensor_copy(h0[i][:], pbt[:])

        # ================= phase 4: iter-0 hop-1 (the big one) =================
        for i in range(nt):
            x1T = big.tile([D, K * P], F32, tag="x1T")
            for m in range(K):
                ev2 = gat.tile([P, K * D], F32, tag="ev2")
                for n in range(K):
                    gather(
                        ev2[:, n * D : (n + 1) * D], ent_d[:],
                        e2t[i][:, m * K + n : m * K + n + 1],
                    )
                wev = work.tile([P, K, D], F32, tag="wev1")
                nc.vector.tensor_tensor(
                    out=wev[:],
                    in0=ev2[:].rearrange("p (n d) -> p n d", n=K),
                    in1=esc1[i][:, m * K : (m + 1) * K].broadcast_to([P, K, D]),
                    op=ALU.mult,
                )
                agg = work.tile([P, D], F32, tag="agg1")
                nc.vector.tensor_reduce(
                    out=agg[:],
                    in_=wev[:].rearrange("p n d -> p d n"),
                    axis=mybir.AxisListType.X,
                    op=ALU.add,
                )
                xm = work.tile([P, D], F32, tag="xm")
                nc.vector.scalar_tensor_tensor(
                    out=xm[:], in0=agg[:], scalar=rec1[i][:, m : m + 1],
                    in1=ev1[i][:, m * D : (m + 1) * D], op0=ALU.mult, op1=ALU.add,
                )
                pst = psT.tile([D, P], F32, tag="pst")
                nc.tensor.transpose(pst[:], xm[:], ident[:])
                nc.vector.tensor_copy(x1T[:, m * P : (m + 1) * P], pst[:])

            h1T = big.tile([D, K * P], F32, tag="h1T")
            for j in range(K * P // 512):
                pm = psM.tile([D, 512], F32, tag="mm")
                nc.tensor.matmul(
                    pm[:], lhsT=wt_sb[:], rhs=x1T[:, j * 512 : (j + 1) * 512],
                    start=True, stop=True,
                )
                nc.scalar.activation(
                    h1T[:, j * 512 : (j + 1) * 512], pm[:], AF.Sigmoid,
                    bias=bias_sb[:, 0:1],
                )
            for m in range(K):
                pbt = psB.tile([P, D], F32, tag="pbt")
                nc.tensor.transpose(pbt[:], h1T[:, m * P : (m + 1) * P], ident[:D, :D])
                nc.vector.tensor_copy(h1[i][:, m * D : (m + 1) * D], pbt[:])

        # ================= phase 6: iter-1 hop-0 + final score =================
        for i in range(nt):
            wev = work.tile([P, K, D], F32, tag="wevf")
            nc.vector.tensor_tensor(
                out=wev[:],
                in0=h1[i][:].rearrange("p (n d) -> p n d", n=K),
                in1=esc0[i][:].broadcast_to([P, K, D]),
                op=ALU.mult,
            )
            agg = work.tile([P, D], F32, tag="aggf")
            nc.vector.tensor_reduce(
                out=agg[:],
                in_=wev[:].rearrange("p n d -> p d n"),
                axis=mybir.AxisListType.X,
                op=ALU.add,
            )
            xf = work.tile([P, D], F32, tag="xf")
            nc.vector.scalar_tensor_tensor(
                out=xf[:], in0=agg[:], scalar=rec0[i][:, 0:1], in1=h0[i][:],
                op0=ALU.mult, op1=ALU.add,
            )
            pst = psT.tile([D, P], F32, tag="pst")
            nc.tensor.transpose(pst[:], xf[:], ident[:])
            nc.vector.tensor_copy(xfT[:, i * P : (i + 1) * P], pst[:])

        pmf = psM.tile([D, bl], F32, tag="mm")
        nc.tensor.matmul(pmf[:], lhsT=wt_sb[:], rhs=xfT[:], start=True, stop=True)
        fT = work.tile([D, bl], F32, tag="fT")
        nc.scalar.activation(fT[:], pmf[:], AF.Tanh, bias=bias_sb[:, 0:1])
        prod = work.tile([D, bl], F32, tag="prod")
        nc.vector.tensor_mul(prod[:], fT[:], userT[:])
        pr = psM.tile([1, bl], F32, tag="mm")
        nc.tensor.matmul(pr[:], lhsT=ones64[:], rhs=prod[:], start=True, stop=True)
        out_sb = work.tile([1, bl], F32, tag="out_sb")
        nc.scalar.activation(out_sb[:], pr[:], AF.Sigmoid)
        nc.sync.dma_start(out=out_d[:].rearrange("(one b) -> one b", one=1), in_=out_sb[:])

    nc.finalize()
    return nc


_program_cache = {}


def _get_program(total=TOTAL, bl=BL):
    key = (total, bl)
    if key not in _program_cache:
        _program_cache[key] = build_program(total, bl)
    return _program_cache[key]


def make_in_maps(u, v, adj_ent, adj_rel, entity_embed, rel_embed, W, b, n_cores=N_CORES):
    bl = u.shape[0] // n_cores
    ae32 = np.ascontiguousarray(adj_ent.astype(np.int32))
    ar32 = np.ascontiguousarray(adj_rel.astype(np.int32))
    ent = np.ascontiguousarray(entity_embed.astype(np.float32))
    relT = np.ascontiguousarray(rel_embed.astype(np.float32).T)
    wt = np.ascontiguousarray(W.astype(np.float32).T)
    bias = np.ascontiguousarray(b.astype(np.float32))
    u32 = u.astype(np.int32)
    v32 = v.astype(np.int32)
    return [
        {
            "u32": np.ascontiguousarray(u32[c * bl : (c + 1) * bl]),
            "v32": np.ascontiguousarray(v32[c * bl : (c + 1) * bl]),
            "adj_ent32": ae32,
            "adj_rel32": ar32,
            "ent": ent,
            "relT": relT,
            "Wt": wt,
            "bias": bias,
        }
        for c in range(n_cores)
    ]


def kernel(u, v, adj_ent, adj_rel, entity_embed, rel_embed, W, b, **run_kwargs):
    u = np.asarray(u)
    v = np.asarray(v)
    nc = _get_program(np.asarray(entity_embed).shape[0], u.shape[0] // N_CORES)
    in_maps = make_in_maps(
        u, v, np.asarray(adj_ent), np.asarray(adj_rel),
        np.asarray(entity_embed), np.asarray(rel_embed), np.asarray(W), np.asarray(b),
    )
    res = run_bass_kernel_spmd(nc, in_maps, core_ids=list(range(N_CORES)), **run_kwargs)
    out = np.concatenate([res.results[c]["out"] for c in range(N_CORES)])
    if run_kwargs.get("trace"):
        return out, res
    return out

